# revision 47
# baseline (speedup 1.0000x reference)
"""Causal multi-head attention (B=2, S=2048, H=1024, 16 heads, hd=64) on 8
Trainium2 NeuronCores.

Sharding: batch x head-group. Core c handles batch c//4 and the 4 heads
4*(c%4)..4*(c%4)+3 (a 256-wide column slice of Q/K/V). Each core computes its
heads' contribution to the output projection (row-parallel Wo); the host sums
the 4 bf16 partials per batch and adds bo.

Per-core kernel:
  phase 1: qT/kT mc0 = W.T-slice.T @ xT (+bias via K=1 matmul) and v chunks
           0-7, interleaved by 512-token block so the PE never waits on the
           xt DMA tail. x/Wq/Wk/Wv are bf16 (fast weight loads); scores and
           the output projection run in float32r. xT stays RESIDENT in SBUF
           (no re-DMA later); q/k mc1 and v chunks 8-15 are deferred into
           section A as PE filler.
  phase 2: heads processed in PAIRS (even head on partitions 0-63, odd on
           64-127), section A = heads 0/1, section B = heads 2/3. Per
           (qb, pair) slot, a 2-unit-staggered software pipeline over
           128-wide k-blocks:
             unit kb: scoresT for both heads as two K=64 matmuls in DISJOINT
             PE row halves (they run CONCURRENTLY in the array), one fused
             staircase-mask add on DVE for diagonal kbs (sliced to the
             masked column range), one [128,2,512] exp -> bf16 on ACT, then
             the ctx matmuls of unit kb-2 (K=128; bf16 vaug carries a ones
             column so row 64 is the softmax denominator).
           The last two ctx pairs of each slot run one unit INTO the next
           slot (behind its first scores) so their exp latency never stalls
           the in-order PE queue at slot boundaries. Epilogue per slot: one
           DVE copy evacuates ctx+denominators PSUM->SBUF (frees the PSUM
           banks fast), ACT computes 1/d = exp(-ln d) off-path, and the PE
           broadcast + DVE normalize run deferred 3 units into the next
           slot. PE filler singles (v/qk projection chains in section A,
           outproj chunks in section B) are popped between units to keep
           the PE dense -- without them the ACT-paced attention stream lets
           the HAM activity monitor re-throttle the PE clock to 1.2 GHz.
           Slot order starts with a deep-pipeline slot per section
           (A: 1,0,2,3; B: 3,2,1,0) so all-diagonal qb=0 slots never lead.
  phase 3: out_partial[tok,1024] = ctxT.T @ WoT-slice in f32r, written to
           DRAM as bf16 (emitted as section-B fillers).
"""
import ml_dtypes
import numpy as np

import concourse.bass as bass
import concourse.mybir as mybir
import concourse.tile as tile
from concourse.bass import ts
from concourse.bass_utils import run_bass_kernel_spmd

B, S, H, NH, HD = 2, 2048, 1024, 16, 64
NCORES = 8
HPC = 4            # heads per core
HSW = HPC * HD     # 256: head-slice width
F32 = mybir.dt.float32
F32R = mybir.dt.float32r
NEG = -1.0e9


def _split_multi_waits(nc) -> int:
    """This walrus accepts at most ONE sync wait per instruction. Split any
    multi-wait instruction into single-wait NOPs (same engine, just before
    it) + the instruction carrying the last wait. Equivalent semantics:
    waits run in program order on the engine's queue."""
    n = 0
    for f in nc.m.functions:
        for blk in f.blocks:
            new_insts = []
            for inst in blk.instructions:
                si = inst.sync_info
                if si is not None and si.on_wait and len(si.on_wait) > 1:
                    waits = list(si.on_wait)
                    for i, w in enumerate(waits[:-1]):
                        new_insts.append(mybir.InstNoOp(
                            name=f"{inst.name}-ws{i}",
                            engine=inst.engine,
                            bass_nofuse=True,
                            sync_info=mybir.SyncInfo(on_wait=[w], on_update=[]),
                        ))
                        n += 1
                    si.on_wait = [waits[-1]]
                new_insts.append(inst)
            blk.instructions[:] = new_insts
    return n


def _build():
    nc = bass.Bass()
    xt_d = nc.dram_tensor("xt", [H, S], mybir.dt.bfloat16, kind="ExternalInput")
    wq_d = nc.dram_tensor("wq", [H, HSW], mybir.dt.bfloat16, kind="ExternalInput")
    wk_d = nc.dram_tensor("wk", [H, HSW], mybir.dt.bfloat16, kind="ExternalInput")
    wv_d = nc.dram_tensor("wv", [H, HSW], mybir.dt.bfloat16, kind="ExternalInput")
    wo_d = nc.dram_tensor("wo", [HSW, H], F32R, kind="ExternalInput")
    bqkv_d = nc.dram_tensor("bqkv", [3, HSW], F32R, kind="ExternalInput")
    ones_d = nc.dram_tensor("onesd", [1, 512], F32R, kind="ExternalInput")
    vones_d = nc.dram_tensor("vonesd", [128, 64], mybir.dt.bfloat16,
                             kind="ExternalInput")
    mb_d = nc.dram_tensor("mb", [128, 1280], F32, kind="ExternalInput")
    out_d = nc.dram_tensor("out", [S, H], mybir.dt.bfloat16, kind="ExternalOutput")

    EXP = mybir.ActivationFunctionType.Exp
    NQB = S // 512      # 4 query blocks per head
    NTC = S // 128      # 16 token chunks

    with tile.TileContext(nc) as tc:
        with tc.tile_pool(name="const", bufs=1) as constp, \
             tc.tile_pool(name="persist", bufs=1) as pers:
            wq = constp.tile([128, 8, HSW], mybir.dt.bfloat16)
            wk = constp.tile([128, 8, HSW], mybir.dt.bfloat16)
            wv = constp.tile([128, 8, HSW], mybir.dt.bfloat16)
            wo = constp.tile([128, 2, H], F32R)
            bqkv = constp.tile([1, 3, HSW], F32R)
            # packed staircase mask: tile j at col offset 128*j*(j+1)/2,
            # width 128*(j+1) (cols beyond that are 0 in tile j)
            mb = constp.tile([128, 2, 1280], F32)
            MBOFF = [0, 128, 384, 768]
            ones = constp.tile([1, 512], F32R)

            xt = pers.tile([128, 8, S], mybir.dt.bfloat16)  # resident whole kernel
            qT = pers.tile([128, 2, S], F32R)     # [2 heads x 64 hd, mchunk, tok]
            kT = pers.tile([128, 2, S], F32R)
            vaug = pers.tile([128, 4, NTC, HD + 1], mybir.dt.bfloat16)  # [ktok, head, kchunk, hd|1]
            ctxT = pers.tile([128, 2, S], F32R)   # outproj stationary layout


            # ---- DMA: one big 3D descriptor per 512-token xt block (cheap
            # to issue, the DMA engines spray the strided gather); weights
            # interleaved so the first q/k/v chains unblock early.
            nc.sync.dma_start(out=ones, in_=ones_d[:, :])
            nc.sync.dma_start(
                out=vaug[:, :, :, HD:HD + 1],
                in_=vones_d[:, :].rearrange("p (a b o) -> p a b o", a=4, o=1))
            nc.sync.dma_start(out=wq, in_=wq_d[:, :].rearrange("(c p) n -> p c n", p=128))
            nc.sync.dma_start(
                out=xt[:, :, ts(0, 512)],
                in_=xt_d[:, ts(0, 512)].rearrange("(c p) n -> p c n", p=128))
            nc.sync.dma_start(out=wk, in_=wk_d[:, :].rearrange("(c p) n -> p c n", p=128))
            nc.sync.dma_start(out=wv, in_=wv_d[:, :].rearrange("(c p) n -> p c n", p=128))
            nc.sync.dma_start(out=bqkv, in_=bqkv_d[:, :].rearrange("(o r) n -> o r n", o=1))
            for nb in range(1, NQB):
                nc.sync.dma_start(
                    out=xt[:, :, ts(nb, 512)],
                    in_=xt_d[:, ts(nb, 512)].rearrange("(c p) n -> p c n", p=128))
            for hi in range(2):
                nc.sync.dma_start(out=mb[:, hi, :], in_=mb_d[:, :])
            nc.sync.dma_start(out=wo, in_=wo_d[:, :].rearrange("(c p) n -> p c n", p=128))

            # ---- phase 1: q/k mc0 + v chunks 0-7, interleaved by nb so the
            # PE never waits on the xt DMA tail (v chunks 0-7 only touch
            # nb0/nb1 columns). q/k mc1 and v chunks 8-15 are deferred as
            # section-A fillers. ----
            with tc.tile_pool(name="ps1", bufs=3, space="PSUM") as ps1, \
                 tc.tile_pool(name="ps1v", bufs=3, space="PSUM") as ps1v:
                def v_chunk(t, pool, tag):
                    ps = pool.tile([128, HSW], F32, tag=tag, name="vps")
                    for kc in range(8):
                        nc.tensor.matmul(ps, xt[:, kc, ts(t, 128)], wv[:, kc, :],
                                         start=(kc == 0), stop=False)
                    nc.tensor.matmul(ps, ones[0:1, 0:128], bqkv[0:1, 2, :],
                                     start=False, stop=True)
                    nc.vector.tensor_copy(
                        out=vaug[:, :, t, 0:HD],
                        in_=ps[:, :].rearrange("p (h d) -> p h d", h=HPC))

                for nb in range(NQB):
                    for w, brow, dst in ((wq, 0, qT), (wk, 1, kT)):
                        ps = ps1.tile([128, 512], F32, tag="ps1")
                        for kc in range(8):
                            nc.tensor.matmul(ps, w[:, kc, 0:128],
                                             xt[:, kc, ts(nb, 512)],
                                             start=(kc == 0), stop=False)
                        nc.tensor.matmul(ps, bqkv[0:1, brow, 0:128],
                                         ones[0:1, 0:512], start=False, stop=True)
                        nc.vector.tensor_copy(out=dst[:, 0, ts(nb, 512)], in_=ps)
                    if nb < 2:
                        for t in range(4 * nb, 4 * nb + 4):
                            v_chunk(t, ps1v, "psv")

            # ---- phase 2+3: paired-head attention pipeline ----
            with tc.tile_pool(name="pss", bufs=2, space="PSUM") as pss, \
                 tc.tile_pool(name="psc", bufs=1, space="PSUM") as psc, \
                 tc.tile_pool(name="pso", bufs=2, space="PSUM") as pso, \
                 tc.tile_pool(name="attnp", bufs=3) as attnp, \
                 tc.tile_pool(name="epip", bufs=2) as epip, \
                 tc.tile_pool(name="outp", bufs=3) as outp:

                # -- filler step machinery: each step emits ~1 PE matmul --
                def v_chunk_steps(t):
                    st = {}
                    steps = []
                    def mk(kc):
                        def run():
                            if kc == 0:
                                st['ps'] = pso.tile([128, HSW], F32,
                                                    tag="pso", name="vps")
                            nc.tensor.matmul(st['ps'], xt[:, kc, ts(t, 128)],
                                             wv[:, kc, :],
                                             start=(kc == 0), stop=False)
                        return run
                    for kc in range(8):
                        steps.append(mk(kc))
                    def fin():
                        nc.tensor.matmul(st['ps'], ones[0:1, 0:128],
                                         bqkv[0:1, 2, :], start=False,
                                         stop=True)
                        nc.vector.tensor_copy(
                            out=vaug[:, :, t, 0:HD],
                            in_=st['ps'][:, :].rearrange("p (h d) -> p h d",
                                                         h=HPC))
                    steps.append(fin)
                    return steps

                def qk_mc1_steps(w, brow, dst, nb):
                    st = {}
                    steps = []
                    def mk(kc):
                        def run():
                            if kc == 0:
                                st['ps'] = pso.tile([128, 512], F32,
                                                    tag="pso", name="pso")
                            nc.tensor.matmul(st['ps'], w[:, kc, 128:256],
                                             xt[:, kc, ts(nb, 512)],
                                             start=(kc == 0), stop=False)
                        return run
                    for kc in range(8):
                        steps.append(mk(kc))
                    def fin():
                        nc.tensor.matmul(st['ps'], bqkv[0:1, brow, 128:256],
                                         ones[0:1, 0:512], start=False,
                                         stop=True)
                        nc.vector.tensor_copy(out=dst[:, 1, ts(nb, 512)],
                                              in_=st['ps'])
                    steps.append(fin)
                    return steps

                def outproj_steps(qb):
                    steps = []
                    for t in range(4 * qb, 4 * qb + 4):
                        st = {}
                        def mk(t, n2, st=None):
                            def run():
                                if n2 == 0:
                                    st['osb'] = outp.tile([128, H],
                                                          mybir.dt.bfloat16,
                                                          tag="osb", name="osb")
                                ops = pso.tile([128, 512], F32, tag="pso",
                                               name="opso")
                                nc.tensor.matmul(ops, ctxT[:, 0, ts(t, 128)],
                                                 wo[:, 0, ts(n2, 512)],
                                                 start=True, stop=False)
                                nc.tensor.matmul(ops, ctxT[:, 1, ts(t, 128)],
                                                 wo[:, 1, ts(n2, 512)],
                                                 start=False, stop=True)
                                nc.vector.tensor_copy(out=st['osb'][:, ts(n2, 512)],
                                                      in_=ops)
                                if n2 == 1:
                                    nc.sync.dma_start(out=out_d[ts(t, 128), :],
                                                      in_=st['osb'])
                            return run
                        st = {}
                        steps.append(mk(t, 0, st))
                        steps.append(mk(t, 1, st))
                    return steps

                fillers = []

                reserve = [0]

                def pop_fillers(k):
                    for _ in range(k):
                        if len(fillers) > reserve[0]:
                            fillers.pop(0)()

                def epilogue_act(ctxc):
                    # reciprocal of both heads' denominator rows via
                    # exp(-ln d) on ACT (one [1,2,512] op per stage),
                    # reading the SBUF context copy: off the PE critical path
                    lnr = epip.tile([1, 2, 512], F32, tag="lnr", name="lnr")
                    nc.scalar.activation(out=lnr, in_=ctxc[HD:HD + 1, :, :],
                                         func=mybir.ActivationFunctionType.Ln)
                    rec = epip.tile([1, 2, 512], F32R, tag="rec", name="rec")
                    nc.scalar.activation(out=rec, in_=lnr, func=EXP,
                                         scale=-1.0)
                    return rec

                def epilogue_pe(qb, mc, ctxc, rec):
                    # broadcast 1/d across 64 partitions (PE) + normalize.
                    # Runs DEFERRED, three units into the next slot: the rec
                    # is long since ready, so the bcast never stalls the PE.
                    def run():
                        for hi in range(2):
                            ro = hi * HD
                            bps = pso.tile([128, 512], F32, tag="pso",
                                           name="bps")
                            nc.tensor.matmul(bps[0:HD, :], ones[0:1, 0:HD],
                                             rec[0:1, hi, :],
                                             start=True, stop=True)
                            bsb = epip.tile([HD, 512], F32R, tag="bsb",
                                            name="bsb")
                            nc.vector.tensor_copy(out=bsb, in_=bps[0:HD, :])
                            nc.vector.tensor_mul(
                                out=ctxT[ro:ro + HD, mc, ts(qb, 512)],
                                in0=ctxc[0:HD, hi, :], in1=bsb)
                    return run

                pending = []

                tail = []  # prev slot's last two ctx pairs + recip setup

                def slot(qb, mc, late_fillers=None, last_slot=False):
                    last_kb = 4 * qb + 3
                    cps = psc.tile([128, 2, 512], F32, tag="ctx", name="cps")
                    ets = {}

                    def ctx_pair(kb):
                        for hi in range(2):
                            h = 2 * mc + hi
                            nc.tensor.matmul(cps[0:HD + 1, hi, :],
                                             vaug[:, h, kb, :],
                                             ets[kb][:, hi, :],
                                             start=(kb == 0),
                                             stop=(kb == last_kb))

                    for kb in range(last_kb + 1):
                        sps = pss.tile([128, 2, 512], F32, tag="s",
                                       name="sps")
                        for hi in range(2):
                            ro = hi * HD
                            nc.tensor.matmul(
                                sps[:, hi, :],
                                kT[ro:ro + HD, mc, ts(kb, 128)],
                                qT[ro:ro + HD, mc, ts(qb, 512)],
                                start=True, stop=True)
                        j = kb - 4 * qb
                        if j >= 0:
                            w = 128 * (j + 1)
                            nc.vector.tensor_add(
                                sps[:, :, 0:w], sps[:, :, 0:w],
                                mb[:, :, MBOFF[j]:MBOFF[j] + w])
                        et = attnp.tile([128, 2, 512], mybir.dt.bfloat16,
                                        tag="et", name="et")
                        nc.scalar.activation(out=et, in_=sps, func=EXP,
                                             scale=0.125)
                        ets[kb] = et
                        if kb == 0 and tail:
                            # the previous slot's last two ctx pairs run
                            # AFTER this slot's first scores: their exp
                            # latency hides behind fresh PE work instead of
                            # stalling the in-order PE queue at the boundary
                            tail.pop(0)()
                        if kb == (2 if last_slot else 3):
                            if pending:
                                pending.pop(0)()
                            if late_fillers:
                                # outproj(qb-1) reads ctxT written by the
                                # pending epilogue popped just above: extend
                                # only after that emission
                                fillers.extend(late_fillers)
                        if kb >= 2:
                            ctx_pair(kb - 2)
                        pop_fillers(pop_rate[0])

                    def fin():
                        ctx_pair(last_kb - 1)
                        pop_fillers(1)
                        ctx_pair(last_kb)
                        if last_slot:
                            # nothing reuses these banks: normalize straight
                            # from PSUM, skipping the evacuation copy
                            rec = epilogue_act(cps)
                            pending.append(epilogue_pe(qb, mc, cps, rec))
                            return
                        # free the ctx PSUM banks fast: one copy to SBUF,
                        # then the normalize chain works from the copy
                        ctxc = epip.tile([HD + 1, 2, 512], F32, tag="ctxc",
                                         name="ctxc")
                        nc.vector.tensor_copy(out=ctxc, in_=cps[0:HD + 1, :, :])
                        rec = epilogue_act(ctxc)
                        pending.append(epilogue_pe(qb, mc, ctxc, rec))
                    tail.append(fin)

                pop_rate = [3]
                reserve[0] = 18
                # section A: heads 0/1 (mc=0); fillers: v chunks 8-15 first
                # (consumed by A2+ ctx), then q/k mc1 projections; the nb0
                # chains go LAST and are held in reserve for section B's
                # leading slot (B runs big-slots-first, so nb0's consumer B0
                # is its final slot)
                for t in range(8, NTC):
                    fillers.extend(v_chunk_steps(t))
                # kT is consumed by EVERY section-B slot (causal prefix), so
                # all k-chains must flush in section A; only q-nb0 (read
                # solely by B0, the final slot) can be held in reserve
                for nb in range(NQB):
                    fillers.extend(qk_mc1_steps(wk, 1, kT, nb))
                for nb in (2, 3, 1, 0):
                    fillers.extend(qk_mc1_steps(wq, 0, qT, nb))
                # A1 leads: its first 4 off-diagonal units fill the
                # pipeline before any all-diagonal serial chains (A0) run
                for qb in (1, 0, 2, 3):
                    slot(qb, 0)
                # flush everything except the reserved nb0 chains: section B
                # leads with B3, whose qk-nb3 inputs must be emitted first
                pop_fillers(len(fillers))
                reserve[0] = 0
                pop_rate[0] = 2

                # section B: heads 2/3 (mc=1), BIG SLOTS FIRST so the deep
                # off-diagonal pipeline leads the section (the all-diagonal
                # qb=0 slot's serial mask->exp chains run last, against the
                # outproj tail). Fillers: reserved nb0 chains, then outproj
                # of the previously completed slot.
                prev_qb = None
                for qb in (3, 2, 1, 0):
                    slot(qb, 1,
                         late_fillers=(outproj_steps(prev_qb)
                                       if prev_qb is not None else None),
                         last_slot=(qb == 0))
                    prev_qb = qb
                while tail:
                    tail.pop(0)()
                pop_fillers(len(fillers))
                while pending:
                    pending.pop(0)()
                for fn in outproj_steps(0):
                    fn()

    _split_multi_waits(nc)
    return nc


_NC_CACHE = []


def _get_nc():
    if not _NC_CACHE:
        _NC_CACHE.append(_build())
    return _NC_CACHE[0]


def _staircase_mask() -> np.ndarray:
    """Packed staircase mask: tile j (width 128*(j+1)) at col offset
    128*j*(j+1)/2. mb_j[p, f] = 0 where k<=q, else NEG: allowed iff
    p <= f - 128*j (q = qb*512+f, k = qb*512+128*j+p). Cols beyond
    128*(j+1) of tile j are all-0 and never read."""
    p = np.arange(128)[:, None]
    out = np.empty((128, 1280), dtype=np.float32)
    off = [0, 128, 384, 768]
    for j in range(4):
        w = 128 * (j + 1)
        f = np.arange(w)[None, :]
        out[:, off[j]:off[j] + w] = np.where(p <= f - 128 * j, 0.0, NEG)
    return out


def _in_maps(inputs: dict) -> list[dict]:
    x = np.ascontiguousarray(np.asarray(inputs["hidden_states"], dtype=np.float32))
    Wq = np.asarray(inputs["Wq"], dtype=np.float32)
    Wk = np.asarray(inputs["Wk"], dtype=np.float32)
    Wv = np.asarray(inputs["Wv"], dtype=np.float32)
    Wo = np.asarray(inputs["Wo"], dtype=np.float32)
    bq = np.asarray(inputs["bq"], dtype=np.float32)
    bk = np.asarray(inputs["bk"], dtype=np.float32)
    bv = np.asarray(inputs["bv"], dtype=np.float32)

    xts = [np.ascontiguousarray(x[b].T) for b in range(B)]
    mb = _staircase_mask()
    maps = []
    for c in range(NCORES):
        b, hg = c // 4, c % 4
        hs = slice(hg * HSW, (hg + 1) * HSW)
        maps.append({
            "xt": xts[b].astype(ml_dtypes.bfloat16),
            "wq": np.ascontiguousarray(Wq[hs, :].T).astype(ml_dtypes.bfloat16),
            "wk": np.ascontiguousarray(Wk[hs, :].T).astype(ml_dtypes.bfloat16),
            "wv": np.ascontiguousarray(Wv[hs, :].T).astype(ml_dtypes.bfloat16),
            "wo": np.ascontiguousarray(Wo[:, hs].T),
            "bqkv": np.ascontiguousarray(np.stack([bq[hs], bk[hs], bv[hs]])),
            "onesd": np.ones((1, 512), dtype=np.float32),
            "vonesd": np.ones((128, 64), dtype=ml_dtypes.bfloat16),
            "mb": mb,
        })
    return maps


def run(inputs: dict, **spmd_kwargs):
    """Returns (full_output, BassKernelResults)."""
    nc = _get_nc()
    res = run_bass_kernel_spmd(nc, _in_maps(inputs), list(range(NCORES)),
                               **spmd_kwargs)
    bo = np.asarray(inputs["bo"], dtype=np.float32)
    out = np.empty((B, S, H), dtype=np.float32)
    for b in range(B):
        acc = res.results[4 * b]["out"].astype(np.float32)
        for hg in range(1, 4):
            acc = acc + res.results[4 * b + hg]["out"]
        out[b] = acc + bo
    return out, res


def kernel(**inputs) -> np.ndarray:
    out, _ = run(inputs)
    return out
